# revision 21
# baseline (speedup 1.0000x reference)
"""MLA (DeepSeek-V2-Lite) forward kernel for 8 Trainium2 NeuronCores.

v5: v4 + token-sharded kv projection. The latent/k_pe projection is
head-shared, so v4 computed it identically on all 8 cores (5 of 8
projection blocks replicated). v5 gives each core one 512-token chunk:
it projects kv/k_pe for its own chunk only, RMS-normalizes and ropes it,
then an on-device AllGather (DRAM bounce) assembles the full [c, t]
latent + k_pe on every core while the (head-sharded) q projections for
all 8 chunks stream on the PE in the gather's shadow.

v4: materialized per-head attention. Instead of the absorbed form (scores
and attend both contract over the shared latent C=512), phase 1.5
materializes per-head k_nope[t, 128] = kv_lat @ wkv_b1^T and
v[t, 128] = kv_lat @ wkv_b2 once (tiny matmuls), so both S x S attention
matmuls contract over 128 instead of 512 -- 4x less PE work in attention.
Projections run weight-stationary over a resident xT [DIM, tokens] so every
output lands in [feature, token] layout. Scores are computed TRANSPOSED
(scT[t, s]) via stationary k_nopeT blocks against moving qT_nope, so exp()
writes attn^T directly. Softmax denominators come from a ones-column matmul
over attn^T; the 1/sum scale is folded into the xv PSUM evacuation multiply.
RMSNorm of the latent is computed in the [c, t] layout directly (squares +
ones-column matmul for the partition reduction), so no PE transposes remain.

Sharding: tensor-parallel over heads (2 heads/core), partial wo outputs
summed on the host.
"""

import sys

for _p in ("/opt/trn_rl_repo",):
    if _p not in sys.path:
        sys.path.append(_p)

import numpy as np
import ml_dtypes

import concourse.bacc as bacc
import concourse.tile as tile
import concourse.mybir as mybir
from concourse import bass_utils

BF16 = mybir.dt.bfloat16
F32 = mybir.dt.float32
AF = mybir.ActivationFunctionType

DIM = 2048
H = 16
C = 512          # kv_lora_rank
NOPE = 128
R = 64           # rope dim
V = 128          # v_head_dim
QK = NOPE + R
B = 2
S = 2048
N_CORES = 8
HL = H // N_CORES   # heads per core (2)
P = 128
DT = DIM // P       # 16 K-tiles over model dim
CT = C // P         # 4 c-blocks
NBLK = 8            # projection M-blocks: kv0-3, kpe, ropes, qn0, qn1
CH = 512            # projection token-chunk (one PSUM bank)
NEG = -1.0e30


def _emit_rope_t(nc, pool, ps, pbase, cs4, cs4sw, out_ev, out_od, tag):
    """Rope on transposed layout. ps rows [pbase:pbase+32] = even coeffs,
    [pbase+32:pbase+64] = odd (PSUM f32). cs4 rows 0:32 = cos, 32:64 = sin;
    cs4sw rows 0:32 = sin, 32:64 = cos. Walrus requires equal base
    partitions when BOTH tensor-tensor inputs are SBUF, so the partition
    shift always rides the PSUM operand (exempt) and every SBUF operand
    of an op shares its base."""
    RH = R // 2
    w1 = pool.tile([R, CH], F32, tag=tag + "w1", bufs=1)
    w2 = pool.tile([R, CH], F32, tag=tag + "w2", bufs=1)
    ev = ps[pbase:pbase + RH, :]
    od = ps[pbase + RH:pbase + 2 * RH, :]
    nc.vector.tensor_mul(w1[0:RH, :], ev, cs4[0:RH, :])        # ev*cos @b0
    nc.vector.tensor_mul(w2[0:RH, :], od, cs4sw[0:RH, :])      # od*sin @b0
    nc.vector.tensor_mul(w1[RH:R, :], ev, cs4[RH:R, :])        # ev*sin @b32
    nc.vector.tensor_mul(w2[RH:R, :], od, cs4sw[RH:R, :])      # od*cos @b32
    nc.vector.tensor_sub(out_ev, w1[0:RH, :], w2[0:RH, :])
    nc.vector.tensor_add(out_od, w1[RH:R, :], w2[RH:R, :])


def build_nc(s_per_b=S, n_cores=N_CORES):
    ST = s_per_b // P          # s-tiles per batch (16)
    TT = B * ST                # total token tiles (32)
    NTOK = B * s_per_b         # 4096
    NCH = NTOK // CH           # 8 chunks

    nc = bacc.Bacc("TRN2", target_bir_lowering=False, debug=False,
                   num_devices=n_cores)

    xc_d = nc.dram_tensor("xc", [NCH, P, DT, CH], BF16,
                          kind="ExternalInput").ap()
    xTkv_d = nc.dram_tensor("xTkv", [P, DT, CH], BF16,
                            kind="ExternalInput").ap()
    cs4kv_d = nc.dram_tensor("cs4kv", [P, CH], F32,
                             kind="ExternalInput").ap()
    cs4swkv_d = nc.dram_tensor("cs4swkv", [P, CH], F32,
                               kind="ExternalInput").ap()
    wallkv_d = nc.dram_tensor("wallkv", [P, DT, CT + 1, P], BF16,
                              kind="ExternalInput").ap()
    wall_d = nc.dram_tensor("wall", [P, DT, NBLK - CT - 1, P], BF16,
                            kind="ExternalInput").ap()
    wkvb1T_d = nc.dram_tensor("wkvb1T", [P, CT, HL, NOPE], BF16,
                              kind="ExternalInput").ap()
    wkvb2_d = nc.dram_tensor("wkvb2", [P, CT, HL, V], BF16,
                             kind="ExternalInput").ap()
    woT_d = nc.dram_tensor("woT", [V, HL, DIM], BF16,
                           kind="ExternalInput").ap()
    cs4_d = nc.dram_tensor("cs4", [P, s_per_b], F32, kind="ExternalInput").ap()
    cs4sw_d = nc.dram_tensor("cs4sw", [P, s_per_b], F32,
                             kind="ExternalInput").ap()
    maskA_d = nc.dram_tensor("maskA", [P, HL, 2, P], BF16,
                             kind="ExternalInput").ap()
    maskB_d = nc.dram_tensor("maskB", [P, HL, 2, P], BF16,
                             kind="ExternalInput").ap()
    y_d = nc.dram_tensor("y", [NTOK, DIM], BF16, kind="ExternalOutput").ap()

    with tile.TileContext(nc) as tc:
        with tc.tile_pool(name="static", bufs=1) as st:
            maskA_sb = st.tile([P, HL, 2, P], BF16)
            maskB_sb = st.tile([P, HL, 2, P], BF16)
            ones_col = st.tile([P, 1], BF16)
            nc.vector.memset(ones_col, 1.0)
            eps_sb = st.tile([1, 1], F32)
            nc.vector.memset(eps_sb, 1e-6)

            # residents
            kv_latT_sb = st.tile([P, NCH, CT, CH], BF16)  # [c%128, g, kc, t']
            k_peT_sb = st.tile([R, NTOK], BF16)          # [r', t]
            qT_pe_sb = st.tile([R, HL, NTOK], BF16)      # [r', h, t]
            qT_nope_sb = st.tile([P, HL, NTOK], BF16)    # [d, h, t]
            wkvb1T_sb = st.tile([P, CT, HL, NOPE], BF16)
            wkvb2_sb = st.tile([P, CT, HL, V], BF16)
            woT_sb = st.tile([V, HL, DIM], BF16)

            # ================= PHASE 1: projections =================
            # Stage A: kv latent + k_pe for THIS core's 512-token chunk
            # only, then AllGather the packed result; the q projections
            # for all 8 chunks (Stage B) run in the gather's shadow.
            with tc.tile_pool(name="dram", bufs=1, space="DRAM") as dpool, \
                 tc.tile_pool(name="p1", bufs=1) as p1, \
                 tc.tile_pool(name="p1ps", bufs=1, space="PSUM") as p1ps:
                cc_in = dpool.tile([P, CT + 1, CH], BF16)
                cc_out = dpool.tile([n_cores, P, CT + 1, CH], BF16)
                xkv = p1.tile([P, DT, CH], BF16)
                nc.sync.dma_start(out=xkv, in_=xTkv_d)
                cs4kv = p1.tile([P, CH], F32)
                nc.gpsimd.dma_start(out=cs4kv, in_=cs4kv_d)
                cs4swkv = p1.tile([P, CH], F32)
                nc.gpsimd.dma_start(out=cs4swkv, in_=cs4swkv_d)
                wallkv_sb = p1.tile([P, DT, CT + 1, P], BF16)
                nc.gpsimd.dma_start(out=wallkv_sb, in_=wallkv_d)
                wall_sb = p1.tile([P, DT, NBLK - CT - 1, P], BF16)
                nc.sync.dma_start(out=wall_sb, in_=wall_d)

                slab = p1.tile([P, CT + 1, CH], BF16)
                for blk in range(CT + 1):
                    ps = p1ps.tile([P, CH], F32, tag="proj", bufs=3)
                    for kd in range(DT):
                        nc.tensor.matmul(ps, wallkv_sb[:, kd, blk, :],
                                         xkv[:, kd, :],
                                         start=(kd == 0),
                                         stop=(kd == DT - 1))
                    if blk < CT:
                        # kv latent block -> raw bf16 (normalized below)
                        nc.vector.tensor_copy(out=slab[:, blk, :], in_=ps)
                    else:
                        # k_pe rows 0:64 = [ev|od]; rope it
                        _emit_rope_t(nc, p1, ps, 0, cs4kv, cs4swkv,
                                     slab[0:R // 2, CT, :],
                                     slab[R // 2:R, CT, :], "kpe")

                # RMSNorm of the local latent chunk, in [c, t] layout:
                # per-token sum of squares via ones-column matmul.
                sq = p1.tile([P, CT, CH], BF16, tag="sq", bufs=1)
                ssum_ps = p1ps.tile([1, CH], F32, tag="ssum", bufs=1)
                for kc in range(CT):
                    nc.scalar.activation(out=sq[:, kc], in_=slab[:, kc, :],
                                         func=AF.Square)
                    nc.tensor.matmul(ssum_ps, ones_col, sq[:, kc],
                                     start=(kc == 0), stop=(kc == CT - 1))
                rstd = p1.tile([1, CH], F32, tag="rstd", bufs=1)
                nc.scalar.activation(out=rstd, in_=ssum_ps, func=AF.Sqrt,
                                     bias=eps_sb, scale=1.0 / C)
                nc.vector.reciprocal(rstd, rstd)
                rbc = p1.tile([P, CH], F32, tag="rbc", bufs=1)
                nc.gpsimd.partition_broadcast(rbc, rstd)
                rbc16 = p1.tile([P, CH], BF16, tag="rbc16", bufs=1)
                nc.vector.tensor_copy(out=rbc16, in_=rbc)
                for kc in range(CT):
                    nc.vector.tensor_mul(slab[:, kc, :], slab[:, kc, :],
                                         rbc16)

                nc.scalar.dma_start(out=maskA_sb, in_=maskA_d)
                nc.scalar.dma_start(out=maskB_sb, in_=maskB_d)
                nc.scalar.dma_start(out=wkvb1T_sb, in_=wkvb1T_d)
                nc.scalar.dma_start(out=wkvb2_sb, in_=wkvb2_d)
                nc.scalar.dma_start(out=woT_sb, in_=woT_d)
                nc.gpsimd.dma_start(out=cc_in, in_=slab)
                nc.gpsimd.collective_compute(
                    "AllGather", mybir.AluOpType.bypass,
                    replica_groups=[list(range(n_cores))],
                    ins=[cc_in.opt()], outs=[cc_out.opt()])
                for g in range(n_cores):
                    nc.scalar.dma_start(
                        out=kv_latT_sb[:, g], in_=cc_out[g, :, 0:CT, :])
                    nc.scalar.dma_start(
                        out=k_peT_sb[:, g * CH:(g + 1) * CH],
                        in_=cc_out[g, 0:R, CT, :])

                # Stage B: head-sharded q projections for all 8 chunks.
                for ch in range(NCH):
                    c0 = ch * CH
                    pos0 = (ch % (NCH // B)) * CH  # position within batch
                    xch = p1.tile([P, DT, CH], BF16, tag="xch", bufs=2)
                    nc.sync.dma_start(out=xch, in_=xc_d[ch])
                    cs4 = p1.tile([P, CH], F32, tag="cs4", bufs=2)
                    nc.sync.dma_start(out=cs4, in_=cs4_d[:, pos0:pos0 + CH])
                    cs4sw = p1.tile([P, CH], F32, tag="cs4sw", bufs=2)
                    nc.sync.dma_start(out=cs4sw,
                                      in_=cs4sw_d[:, pos0:pos0 + CH])

                    for blk in range(NBLK - CT - 1):
                        ps = p1ps.tile([P, CH], F32, tag="proj", bufs=3)
                        for kd in range(DT):
                            nc.tensor.matmul(ps, wall_sb[:, kd, blk, :],
                                             xch[:, kd, :],
                                             start=(kd == 0),
                                             stop=(kd == DT - 1))
                        if blk == 0:
                            # q ropes: rows [h0(ev|od) | h1(ev|od)]
                            for h in range(HL):
                                _emit_rope_t(
                                    nc, p1, ps, h * R, cs4, cs4sw,
                                    qT_pe_sb[0:R // 2, h, c0:c0 + CH],
                                    qT_pe_sb[R // 2:R, h, c0:c0 + CH],
                                    f"qpe{h}")
                        else:
                            h = blk - 1
                            nc.vector.tensor_copy(
                                out=qT_nope_sb[:, h, c0:c0 + CH], in_=ps)

            # ========== PHASE 1.5: materialize per-head k_nope, v ==========
            with tc.tile_pool(name="res2", bufs=1) as res2:
                knopeT_sb = res2.tile([P, HL, NTOK], BF16)   # [d, h, t]
                v_sb = res2.tile([P, TT, HL, V], BF16)       # [t%128, tj, h, v]
                with tc.tile_pool(name="p15ps", bufs=1, space="PSUM") as p15ps:
                    for h in range(HL):
                        for ch in range(NCH):
                            c0 = ch * CH
                            ps = p15ps.tile([P, CH], F32, tag="kn", bufs=2)
                            for kc in range(CT):
                                nc.tensor.matmul(
                                    ps, wkvb1T_sb[:, kc, h, :],
                                    kv_latT_sb[:, ch, kc, :],
                                    start=(kc == 0), stop=(kc == CT - 1))
                            nc.vector.tensor_copy(
                                out=knopeT_sb[:, h, c0:c0 + CH], in_=ps)
                    for tj in range(TT):
                        ps = p15ps.tile([P, HL * V], F32, tag="v", bufs=2)
                        tg, tp = tj // (CH // P), (tj % (CH // P)) * P
                        for kc in range(CT):
                            nc.tensor.matmul(
                                ps, kv_latT_sb[:, tg, kc, tp:tp + P],
                                wkvb2_sb[:, kc],
                                start=(kc == 0), stop=(kc == CT - 1))
                        nc.vector.tensor_copy(out=v_sb[:, tj], in_=ps)

                # ============ PHASE 2: attention + output ============
                with tc.tile_pool(name="p2", bufs=1) as p2, \
                     tc.tile_pool(name="p2ps", bufs=1, space="PSUM") as p2ps:
                    pairs = [(b, pr) for b in range(B) for pr in range(ST // 2)]

                    def emit_wo(outT_sb, gi0):
                        # wo of the PREVIOUS pair: emitted one pair late so
                        # its PE matmuls fill the exp-wait hole instead of
                        # stalling behind the recip/broadcast/evac chain.
                        for tt in range(2):
                            gi = gi0 + tt
                            y_sb = p2.tile([P, DIM], BF16, tag="ysb", bufs=2,
                                           name="y_sb")
                            for m0 in range(0, DIM, 512):
                                y_ps = p2ps.tile([P, 512], F32, tag="yps",
                                                 bufs=2, name="y_ps")
                                for h in range(HL):
                                    nc.tensor.matmul(
                                        y_ps, outT_sb[:, h, tt, :],
                                        woT_sb[:, h, m0:m0 + 512],
                                        start=(h == 0), stop=(h == HL - 1))
                                nc.scalar.copy(out=y_sb[:, m0:m0 + 512],
                                               in_=y_ps)
                            nc.sync.dma_start(
                                out=y_d[gi * P:(gi + 1) * P, :], in_=y_sb)

                    prev_wo = None
                    for b, pr in pairs:
                        r0 = 2 * pr
                        r1 = r0 + 1
                        gi0 = b * ST + r0
                        scol = gi0 * P          # 256 token columns
                        nj = r1 + 1

                        attnT = p2.tile([P, ST, HL, 2, P], BF16, tag="attnT",
                                        bufs=2)
                        for j in range(nj):
                            tcol = (b * ST + j) * P
                            scps = p2ps.tile([P, HL, 2, P], F32, tag="sc",
                                             bufs=4)
                            # rope matmul first, full tile with start=True:
                            # start clears has_written BANK-wide, so the
                            # shared-rope accumulate must precede the
                            # per-head region matmuls.
                            nc.tensor.matmul(
                                scps,
                                k_peT_sb[:, tcol:tcol + P],
                                qT_pe_sb[:, :, scol:scol + 2 * P],
                                start=True, stop=False)
                            for h in range(HL):
                                nc.tensor.matmul(
                                    scps[:, h],
                                    knopeT_sb[:, h, tcol:tcol + P],
                                    qT_nope_sb[:, h, scol:scol + 2 * P],
                                    start=False, stop=True)
                            if j == r0:
                                nc.vector.tensor_add(scps, scps, maskA_sb)
                            elif j == r1:
                                nc.vector.tensor_add(scps, scps, maskB_sb)
                            nc.scalar.activation(
                                out=attnT[:, j], in_=scps, func=AF.Exp)

                        if prev_wo is not None:
                            emit_wo(*prev_wo)

                        sume_ps = p2ps.tile([1, HL, 2, P], F32, tag="sume",
                                            bufs=1)
                        for j in range(nj):
                            nc.tensor.matmul(
                                sume_ps, ones_col, attnT[:, j],
                                start=(j == 0), stop=(j == nj - 1))
                        recip = p2.tile([1, HL, 2, P], F32, tag="recip",
                                        bufs=2)
                        nc.vector.reciprocal(recip, sume_ps)
                        rbc2 = p2.tile([P, HL, 2, P], F32, tag="rbc2", bufs=2)
                        nc.gpsimd.partition_broadcast(rbc2, recip)

                        xv_ps = p2ps.tile([V, HL, 2, P], F32, tag="xv",
                                          bufs=1)
                        for h in range(HL):
                            for j in range(nj):
                                nc.tensor.matmul(
                                    xv_ps[:, h],
                                    v_sb[:, b * ST + j, h, :],
                                    attnT[:, j, h],
                                    start=(j == 0), stop=(j == nj - 1))
                        outT_sb = p2.tile([V, HL, 2, P], BF16, tag="outTsb",
                                          bufs=2)
                        nc.vector.tensor_mul(outT_sb, xv_ps, rbc2)
                        prev_wo = (outT_sb, gi0)

                    emit_wo(*prev_wo)

    nc.compile()
    return nc


def _deinterleave(w):
    """[64, DIM] interleaved rope rows -> [ev(32) | od(32)]."""
    return np.concatenate([w[0::2], w[1::2]], axis=0)


def shard_inputs(x, freqs_cis, wq, wkv_a, wkv_b, wo, kv_norm_w,
                 s_per_b=S, n_cores=N_CORES):
    bf16 = ml_dtypes.bfloat16
    scale = np.float32(QK ** -0.5)

    xf = np.asarray(x, np.float32).reshape(B * s_per_b, DIM)
    xT = xf.T.astype(bf16)                                 # [DIM, NTOK]
    NCH = B * s_per_b // CH
    DT_ = DIM // P
    # chunk-major SBUF image: [NCH, P, DT, CH]
    xc = np.ascontiguousarray(
        xT.reshape(DT_, P, NCH, CH).transpose(2, 1, 0, 3))

    fc = np.asarray(freqs_cis, np.float32)
    cosT = np.ascontiguousarray(fc[:, :, 0].T)             # [32, S]
    sinT = np.ascontiguousarray(fc[:, :, 1].T)
    cs4 = np.concatenate([cosT, sinT, cosT, sinT], axis=0)     # [128, S]
    cs4sw = np.concatenate([sinT, cosT, sinT, cosT], axis=0)

    wqf = np.asarray(wq, np.float32)                       # [H*QK, DIM]
    wkva = np.asarray(wkv_a, np.float32)                   # [C+R, DIM]
    wkvb = np.asarray(wkv_b, np.float32).reshape(H, NOPE + V, C)
    wof = np.asarray(wo, np.float32)                       # [DIM, H*V]
    wn = np.asarray(kv_norm_w, np.float32)                 # [C]

    kpe_blk = np.concatenate(
        [_deinterleave(wkva[C:C + R]), np.zeros((R, DIM), np.float32)], axis=0)

    ii = np.arange(P)
    tri = np.where(ii[:, None] <= ii[None, :], 0.0, NEG).astype(np.float32)
    maskA = np.zeros((P, HL, 2, P), np.float32)
    maskA[:, :, 0, :] = tri[:, None, :]
    maskB = np.full((P, HL, 2, P), NEG, np.float32)
    maskB[:, :, 1, :] = tri[:, None, :]
    maskA = maskA.astype(bf16)
    maskB = maskB.astype(bf16)

    in_maps = []
    for c in range(n_cores):
        h0 = c * HL
        ckv0 = c * CH                       # this core's kv token chunk
        pos0 = (c % (B * s_per_b // CH // B)) * CH  # position within batch
        wq_c = wqf.reshape(H, QK, DIM)[h0:h0 + HL] * scale  # [HL, QK, DIM]
        ropes = np.concatenate(
            [_deinterleave(wq_c[h, NOPE:]) for h in range(HL)], axis=0)
        blkkv = [wkva[kc * P:(kc + 1) * P] for kc in range(CT)] + [kpe_blk]
        # [blk, m, DIM] -> SBUF image [P, DT, blk, m]
        wallkv = np.ascontiguousarray(
            np.stack(blkkv, axis=0).reshape(CT + 1, P, DT_, P)
            .transpose(3, 2, 0, 1)).astype(bf16)
        blocks = [ropes, wq_c[0, :NOPE], wq_c[1, :NOPE]]
        wall = np.ascontiguousarray(
            np.stack(blocks, axis=0).reshape(NBLK - CT - 1, P, DT_, P)
            .transpose(3, 2, 0, 1)).astype(bf16)

        b1 = (wkvb[h0:h0 + HL, :NOPE, :] * wn[None, None, :])  # [HL,128,C]
        wkvb1T = np.ascontiguousarray(
            b1.transpose(2, 0, 1).reshape(CT, P, HL, NOPE)
            .transpose(1, 0, 2, 3)).astype(bf16)
        b2 = (wkvb[h0:h0 + HL, NOPE:, :] * wn[None, None, :])  # [HL,V,C]
        wkvb2 = np.ascontiguousarray(
            b2.transpose(2, 0, 1).reshape(CT, P, HL, V)
            .transpose(1, 0, 2, 3)).astype(bf16)
        woT_c = np.ascontiguousarray(
            wof[:, h0 * V:(h0 + HL) * V].T.reshape(HL, V, DIM)
            .transpose(1, 0, 2)).astype(bf16)              # [V, HL, DIM]
        in_maps.append({
            "xc": xc,
            "xTkv": np.ascontiguousarray(xc[c]),
            "cs4kv": np.ascontiguousarray(cs4[:, pos0:pos0 + CH]),
            "cs4swkv": np.ascontiguousarray(cs4sw[:, pos0:pos0 + CH]),
            "wallkv": wallkv,
            "wall": wall,
            "wkvb1T": wkvb1T,
            "wkvb2": wkvb2,
            "woT": woT_c,
            "cs4": cs4,
            "cs4sw": cs4sw,
            "maskA": maskA,
            "maskB": maskB,
        })
    return in_maps


_NC_CACHE = {}


def get_nc(s_per_b=S):
    if s_per_b not in _NC_CACHE:
        _NC_CACHE[s_per_b] = build_nc(s_per_b)
    return _NC_CACHE[s_per_b]


def kernel(x, freqs_cis, wq, wkv_a, wkv_b, wo, kv_norm_w, trace=False):
    nc = get_nc(S)
    in_maps = shard_inputs(x, freqs_cis, wq, wkv_a, wkv_b, wo, kv_norm_w)
    res = bass_utils.run_bass_kernel_spmd(
        nc, in_maps, core_ids=list(range(N_CORES)), trace=trace)
    y = res.results[0]["y"].astype(np.float32)
    for i in range(1, N_CORES):
        y += res.results[i]["y"].astype(np.float32)
    out = y.reshape(B, S, DIM)
    if trace:
        kernel.last_exec_time_ns = res.exec_time_ns
        kernel.last_results = res
    return out


# revision 23
# speedup vs baseline: 1.0083x; 1.0083x over previous
"""MLA (DeepSeek-V2-Lite) forward kernel for 8 Trainium2 NeuronCores.

v5: v4 + token-sharded kv projection. The latent/k_pe projection is
head-shared, so v4 computed it identically on all 8 cores (5 of 8
projection blocks replicated). v5 gives each core one 512-token chunk:
it projects kv/k_pe for its own chunk only, RMS-normalizes and ropes it,
then an on-device AllGather (DRAM bounce) assembles the full [c, t]
latent + k_pe on every core while the (head-sharded) q projections for
all 8 chunks stream on the PE in the gather's shadow.

v4: materialized per-head attention. Instead of the absorbed form (scores
and attend both contract over the shared latent C=512), phase 1.5
materializes per-head k_nope[t, 128] = kv_lat @ wkv_b1^T and
v[t, 128] = kv_lat @ wkv_b2 once (tiny matmuls), so both S x S attention
matmuls contract over 128 instead of 512 -- 4x less PE work in attention.
Projections run weight-stationary over a resident xT [DIM, tokens] so every
output lands in [feature, token] layout. Scores are computed TRANSPOSED
(scT[t, s]) via stationary k_nopeT blocks against moving qT_nope, so exp()
writes attn^T directly. Softmax denominators come from a ones-column matmul
over attn^T; the 1/sum scale is folded into the xv PSUM evacuation multiply.
RMSNorm of the latent is computed in the [c, t] layout directly (squares +
ones-column matmul for the partition reduction), so no PE transposes remain.

Sharding: tensor-parallel over heads (2 heads/core), partial wo outputs
summed on the host.
"""

import sys

for _p in ("/opt/trn_rl_repo",):
    if _p not in sys.path:
        sys.path.append(_p)

import numpy as np
import ml_dtypes

import concourse.bacc as bacc
import concourse.tile as tile
import concourse.mybir as mybir
from concourse import bass_utils

BF16 = mybir.dt.bfloat16
F32 = mybir.dt.float32
AF = mybir.ActivationFunctionType

DIM = 2048
H = 16
C = 512          # kv_lora_rank
NOPE = 128
R = 64           # rope dim
V = 128          # v_head_dim
QK = NOPE + R
B = 2
S = 2048
N_CORES = 8
HL = H // N_CORES   # heads per core (2)
P = 128
DT = DIM // P       # 16 K-tiles over model dim
CT = C // P         # 4 c-blocks
NBLK = 8            # projection M-blocks: kv0-3, kpe, ropes, qn0, qn1
CH = 512            # projection token-chunk (one PSUM bank)
NEG = -1.0e30


def _emit_rope_t(nc, pool, ps, pbase, cs4, cs4sw, out_ev, out_od, tag):
    """Rope on transposed layout. ps rows [pbase:pbase+32] = even coeffs,
    [pbase+32:pbase+64] = odd (PSUM f32). cs4 rows 0:32 = cos, 32:64 = sin;
    cs4sw rows 0:32 = sin, 32:64 = cos. Walrus requires equal base
    partitions when BOTH tensor-tensor inputs are SBUF, so the partition
    shift always rides the PSUM operand (exempt) and every SBUF operand
    of an op shares its base."""
    RH = R // 2
    w1 = pool.tile([R, CH], F32, tag=tag + "w1", bufs=1)
    w2 = pool.tile([R, CH], F32, tag=tag + "w2", bufs=1)
    ev = ps[pbase:pbase + RH, :]
    od = ps[pbase + RH:pbase + 2 * RH, :]
    nc.vector.tensor_mul(w1[0:RH, :], ev, cs4[0:RH, :])        # ev*cos @b0
    nc.vector.tensor_mul(w2[0:RH, :], od, cs4sw[0:RH, :])      # od*sin @b0
    nc.vector.tensor_mul(w1[RH:R, :], ev, cs4[RH:R, :])        # ev*sin @b32
    nc.vector.tensor_mul(w2[RH:R, :], od, cs4sw[RH:R, :])      # od*cos @b32
    nc.vector.tensor_sub(out_ev, w1[0:RH, :], w2[0:RH, :])
    nc.vector.tensor_add(out_od, w1[RH:R, :], w2[RH:R, :])


def build_nc(s_per_b=S, n_cores=N_CORES):
    ST = s_per_b // P          # s-tiles per batch (16)
    TT = B * ST                # total token tiles (32)
    NTOK = B * s_per_b         # 4096
    NCH = NTOK // CH           # 8 chunks

    nc = bacc.Bacc("TRN2", target_bir_lowering=False, debug=False,
                   num_devices=n_cores)

    xc_d = nc.dram_tensor("xc", [NCH, P, DT, CH], BF16,
                          kind="ExternalInput").ap()
    xTkv_d = nc.dram_tensor("xTkv", [P, DT, CH], BF16,
                            kind="ExternalInput").ap()
    cs4kv_d = nc.dram_tensor("cs4kv", [P, CH], F32,
                             kind="ExternalInput").ap()
    cs4swkv_d = nc.dram_tensor("cs4swkv", [P, CH], F32,
                               kind="ExternalInput").ap()
    wallkv_d = nc.dram_tensor("wallkv", [P, DT, CT + 1, P], BF16,
                              kind="ExternalInput").ap()
    wall_d = nc.dram_tensor("wall", [P, DT, NBLK - CT - 1, P], BF16,
                            kind="ExternalInput").ap()
    wkvb1T_d = nc.dram_tensor("wkvb1T", [P, CT, HL, NOPE], BF16,
                              kind="ExternalInput").ap()
    wkvb2_d = nc.dram_tensor("wkvb2", [P, CT, HL, V], BF16,
                             kind="ExternalInput").ap()
    woT_d = nc.dram_tensor("woT", [V, HL, DIM], BF16,
                           kind="ExternalInput").ap()
    cs4_d = nc.dram_tensor("cs4", [P, s_per_b], F32, kind="ExternalInput").ap()
    cs4sw_d = nc.dram_tensor("cs4sw", [P, s_per_b], F32,
                             kind="ExternalInput").ap()
    maskA_d = nc.dram_tensor("maskA", [P, HL, 2, P], BF16,
                             kind="ExternalInput").ap()
    maskB_d = nc.dram_tensor("maskB", [P, HL, 2, P], BF16,
                             kind="ExternalInput").ap()
    y_d = nc.dram_tensor("y", [NTOK, DIM], BF16, kind="ExternalOutput").ap()

    with tile.TileContext(nc) as tc:
        with tc.tile_pool(name="static", bufs=1) as st:
            maskA_sb = st.tile([P, HL, 2, P], BF16)
            maskB_sb = st.tile([P, HL, 2, P], BF16)
            ones_col = st.tile([P, 1], BF16)
            nc.vector.memset(ones_col, 1.0)
            eps_sb = st.tile([1, 1], F32)
            nc.vector.memset(eps_sb, 1e-6)

            # residents
            kv_latT_sb = st.tile([P, NCH, CT, CH], BF16)  # [c%128, g, kc, t']
            k_peT_sb = st.tile([R, NTOK], BF16)          # [r', t]
            qT_pe_sb = st.tile([R, HL, NTOK], BF16)      # [r', h, t]
            qT_nope_sb = st.tile([P, HL, NTOK], BF16)    # [d, h, t]
            wkvb1T_sb = st.tile([P, CT, HL, NOPE], BF16)
            wkvb2_sb = st.tile([P, CT, HL, V], BF16)
            woT_sb = st.tile([V, HL, DIM], BF16)

            # ================= PHASE 1: projections =================
            # Stage A: kv latent + k_pe for THIS core's 512-token chunk
            # only, then AllGather the packed result; the q projections
            # for all 8 chunks (Stage B) run in the gather's shadow.
            with tc.tile_pool(name="dram", bufs=1, space="DRAM") as dpool, \
                 tc.tile_pool(name="p1", bufs=1) as p1, \
                 tc.tile_pool(name="p1ps", bufs=1, space="PSUM") as p1ps:
                cc_in = dpool.tile([P, CT + 1, CH], BF16)
                cc_out = dpool.tile([n_cores, P, CT + 1, CH], BF16)
                xkv = p1.tile([P, DT, CH], BF16)
                nc.sync.dma_start(out=xkv, in_=xTkv_d)
                # HAM warmup: keep the PE busy while inputs stream in so
                # Stage A starts at the full 2.4 GHz clock.
                warm = p1.tile([P, P], BF16)
                nc.vector.memset(warm, 0.0)
                warm_ps = p1ps.tile([P, P], F32, tag="warm", bufs=1)
                for _ in range(128):
                    nc.tensor.matmul(warm_ps, warm, warm,
                                     start=True, stop=True)
                cs4kv = p1.tile([P, CH], F32)
                nc.gpsimd.dma_start(out=cs4kv, in_=cs4kv_d)
                cs4swkv = p1.tile([P, CH], F32)
                nc.gpsimd.dma_start(out=cs4swkv, in_=cs4swkv_d)
                wallkv_sb = p1.tile([P, DT, CT + 1, P], BF16)
                nc.gpsimd.dma_start(out=wallkv_sb, in_=wallkv_d)
                wall_sb = p1.tile([P, DT, NBLK - CT - 1, P], BF16)
                nc.sync.dma_start(out=wall_sb, in_=wall_d)

                slab = p1.tile([P, CT + 1, CH], BF16)
                for blk in range(CT + 1):
                    ps = p1ps.tile([P, CH], F32, tag="proj", bufs=3)
                    for kd in range(DT):
                        nc.tensor.matmul(ps, wallkv_sb[:, kd, blk, :],
                                         xkv[:, kd, :],
                                         start=(kd == 0),
                                         stop=(kd == DT - 1))
                    if blk < CT:
                        # kv latent block -> raw bf16 (normalized below)
                        nc.vector.tensor_copy(out=slab[:, blk, :], in_=ps)
                    else:
                        # k_pe rows 0:64 = [ev|od]; rope it
                        _emit_rope_t(nc, p1, ps, 0, cs4kv, cs4swkv,
                                     slab[0:R // 2, CT, :],
                                     slab[R // 2:R, CT, :], "kpe")

                # RMSNorm of the local latent chunk, in [c, t] layout:
                # per-token sum of squares via ones-column matmul.
                sq = p1.tile([P, CT, CH], BF16, tag="sq", bufs=1)
                ssum_ps = p1ps.tile([1, CH], F32, tag="ssum", bufs=1)
                for kc in range(CT):
                    nc.scalar.activation(out=sq[:, kc], in_=slab[:, kc, :],
                                         func=AF.Square)
                    nc.tensor.matmul(ssum_ps, ones_col, sq[:, kc],
                                     start=(kc == 0), stop=(kc == CT - 1))
                rstd = p1.tile([1, CH], F32, tag="rstd", bufs=1)
                nc.scalar.activation(out=rstd, in_=ssum_ps, func=AF.Sqrt,
                                     bias=eps_sb, scale=1.0 / C)
                nc.vector.reciprocal(rstd, rstd)
                rbc = p1.tile([P, CH], F32, tag="rbc", bufs=1)
                nc.gpsimd.partition_broadcast(rbc, rstd)
                rbc16 = p1.tile([P, CH], BF16, tag="rbc16", bufs=1)
                nc.vector.tensor_copy(out=rbc16, in_=rbc)
                for kc in range(CT):
                    nc.vector.tensor_mul(slab[:, kc, :], slab[:, kc, :],
                                         rbc16)

                xkv1 = xkv[0:1, 0:1, 0:1]
                nc.vector.tensor_copy(out=maskA_sb[0:1, 0:1, 0:1, 0:1],
                                      in_=xkv1)
                nc.vector.tensor_copy(out=maskB_sb[0:1, 0:1, 0:1, 0:1],
                                      in_=xkv1)
                nc.vector.tensor_copy(out=wkvb1T_sb[0:1, 0:1, 0:1, 0:1],
                                      in_=xkv1)
                nc.vector.tensor_copy(out=wkvb2_sb[0:1, 0:1, 0:1, 0:1],
                                      in_=xkv1)
                nc.vector.tensor_copy(out=woT_sb[0:1, 0:1, 0:1], in_=xkv1)
                nc.scalar.dma_start(out=maskA_sb, in_=maskA_d)
                nc.scalar.dma_start(out=maskB_sb, in_=maskB_d)
                nc.scalar.dma_start(out=wkvb1T_sb, in_=wkvb1T_d)
                nc.scalar.dma_start(out=wkvb2_sb, in_=wkvb2_d)
                nc.scalar.dma_start(out=woT_sb, in_=woT_d)
                nc.gpsimd.dma_start(out=cc_in, in_=slab)
                nc.gpsimd.collective_compute(
                    "AllGather", mybir.AluOpType.bypass,
                    replica_groups=[list(range(n_cores))],
                    ins=[cc_in.opt()], outs=[cc_out.opt()])
                for g in range(n_cores):
                    nc.scalar.dma_start(
                        out=kv_latT_sb[:, g], in_=cc_out[g, :, 0:CT, :])
                    nc.scalar.dma_start(
                        out=k_peT_sb[:, g * CH:(g + 1) * CH],
                        in_=cc_out[g, 0:R, CT, :])

                # Stage B: head-sharded q projections for all 8 chunks.
                for ch in range(NCH):
                    c0 = ch * CH
                    pos0 = (ch % (NCH // B)) * CH  # position within batch
                    xch = p1.tile([P, DT, CH], BF16, tag="xch", bufs=2)
                    if ch < 2:
                        nc.vector.tensor_copy(out=xch[0:1, 0:1, 0:1],
                                              in_=xkv[0:1, 0:1, 0:1])
                    nc.sync.dma_start(out=xch, in_=xc_d[ch])
                    cs4 = p1.tile([P, CH], F32, tag="cs4", bufs=2)
                    nc.sync.dma_start(out=cs4, in_=cs4_d[:, pos0:pos0 + CH])
                    cs4sw = p1.tile([P, CH], F32, tag="cs4sw", bufs=2)
                    nc.sync.dma_start(out=cs4sw,
                                      in_=cs4sw_d[:, pos0:pos0 + CH])

                    for blk in range(NBLK - CT - 1):
                        ps = p1ps.tile([P, CH], F32, tag="proj", bufs=3)
                        for kd in range(DT):
                            nc.tensor.matmul(ps, wall_sb[:, kd, blk, :],
                                             xch[:, kd, :],
                                             start=(kd == 0),
                                             stop=(kd == DT - 1))
                        if blk == 0:
                            # q ropes: rows [h0(ev|od) | h1(ev|od)]
                            for h in range(HL):
                                _emit_rope_t(
                                    nc, p1, ps, h * R, cs4, cs4sw,
                                    qT_pe_sb[0:R // 2, h, c0:c0 + CH],
                                    qT_pe_sb[R // 2:R, h, c0:c0 + CH],
                                    f"qpe{h}")
                        else:
                            h = blk - 1
                            nc.vector.tensor_copy(
                                out=qT_nope_sb[:, h, c0:c0 + CH], in_=ps)

            # ========== PHASE 1.5: materialize per-head k_nope, v ==========
            with tc.tile_pool(name="res2", bufs=1) as res2:
                knopeT_sb = res2.tile([P, HL, NTOK], BF16)   # [d, h, t]
                v_sb = res2.tile([P, TT, HL, V], BF16)       # [t%128, tj, h, v]
                with tc.tile_pool(name="p15ps", bufs=1, space="PSUM") as p15ps:
                    for h in range(HL):
                        for ch in range(NCH):
                            c0 = ch * CH
                            ps = p15ps.tile([P, CH], F32, tag="kn", bufs=2)
                            for kc in range(CT):
                                nc.tensor.matmul(
                                    ps, wkvb1T_sb[:, kc, h, :],
                                    kv_latT_sb[:, ch, kc, :],
                                    start=(kc == 0), stop=(kc == CT - 1))
                            nc.vector.tensor_copy(
                                out=knopeT_sb[:, h, c0:c0 + CH], in_=ps)
                    for tj in range(TT):
                        ps = p15ps.tile([P, HL * V], F32, tag="v", bufs=2)
                        tg, tp = tj // (CH // P), (tj % (CH // P)) * P
                        for kc in range(CT):
                            nc.tensor.matmul(
                                ps, kv_latT_sb[:, tg, kc, tp:tp + P],
                                wkvb2_sb[:, kc],
                                start=(kc == 0), stop=(kc == CT - 1))
                        nc.vector.tensor_copy(out=v_sb[:, tj], in_=ps)

                # ============ PHASE 2: attention + output ============
                with tc.tile_pool(name="p2", bufs=1) as p2, \
                     tc.tile_pool(name="p2ps", bufs=1, space="PSUM") as p2ps:
                    pairs = [(b, pr) for b in range(B) for pr in range(ST // 2)]

                    def emit_wo(outT_sb, gi0):
                        # wo of the PREVIOUS pair: emitted one pair late so
                        # its PE matmuls fill the exp-wait hole instead of
                        # stalling behind the recip/broadcast/evac chain.
                        for tt in range(2):
                            gi = gi0 + tt
                            y_sb = p2.tile([P, DIM], BF16, tag="ysb", bufs=2,
                                           name="y_sb")
                            for m0 in range(0, DIM, 512):
                                y_ps = p2ps.tile([P, 512], F32, tag="yps",
                                                 bufs=2, name="y_ps")
                                for h in range(HL):
                                    nc.tensor.matmul(
                                        y_ps, outT_sb[:, h, tt, :],
                                        woT_sb[:, h, m0:m0 + 512],
                                        start=(h == 0), stop=(h == HL - 1))
                                nc.scalar.copy(out=y_sb[:, m0:m0 + 512],
                                               in_=y_ps)
                            nc.sync.dma_start(
                                out=y_d[gi * P:(gi + 1) * P, :], in_=y_sb)

                    prev_wo = None
                    for b, pr in pairs:
                        r0 = 2 * pr
                        r1 = r0 + 1
                        gi0 = b * ST + r0
                        scol = gi0 * P          # 256 token columns
                        nj = r1 + 1

                        attnT = p2.tile([P, ST, HL, 2, P], BF16, tag="attnT",
                                        bufs=2)
                        for j in range(nj):
                            tcol = (b * ST + j) * P
                            scps = p2ps.tile([P, HL, 2, P], F32, tag="sc",
                                             bufs=4)
                            # rope matmul first, full tile with start=True:
                            # start clears has_written BANK-wide, so the
                            # shared-rope accumulate must precede the
                            # per-head region matmuls.
                            nc.tensor.matmul(
                                scps,
                                k_peT_sb[:, tcol:tcol + P],
                                qT_pe_sb[:, :, scol:scol + 2 * P],
                                start=True, stop=False)
                            for h in range(HL):
                                nc.tensor.matmul(
                                    scps[:, h],
                                    knopeT_sb[:, h, tcol:tcol + P],
                                    qT_nope_sb[:, h, scol:scol + 2 * P],
                                    start=False, stop=True)
                            if j == r0:
                                nc.vector.tensor_add(scps, scps, maskA_sb)
                            elif j == r1:
                                nc.vector.tensor_add(scps, scps, maskB_sb)
                            nc.scalar.activation(
                                out=attnT[:, j], in_=scps, func=AF.Exp)

                        if prev_wo is not None:
                            emit_wo(*prev_wo)

                        sume_ps = p2ps.tile([1, HL, 2, P], F32, tag="sume",
                                            bufs=1)
                        for j in range(nj):
                            nc.tensor.matmul(
                                sume_ps, ones_col, attnT[:, j],
                                start=(j == 0), stop=(j == nj - 1))
                        recip = p2.tile([1, HL, 2, P], F32, tag="recip",
                                        bufs=2)
                        nc.vector.reciprocal(recip, sume_ps)
                        rbc2 = p2.tile([P, HL, 2, P], F32, tag="rbc2", bufs=2)
                        nc.gpsimd.partition_broadcast(rbc2, recip)

                        xv_ps = p2ps.tile([V, HL, 2, P], F32, tag="xv",
                                          bufs=1)
                        for h in range(HL):
                            for j in range(nj):
                                nc.tensor.matmul(
                                    xv_ps[:, h],
                                    v_sb[:, b * ST + j, h, :],
                                    attnT[:, j, h],
                                    start=(j == 0), stop=(j == nj - 1))
                        outT_sb = p2.tile([V, HL, 2, P], BF16, tag="outTsb",
                                          bufs=2)
                        nc.vector.tensor_mul(outT_sb, xv_ps, rbc2)
                        prev_wo = (outT_sb, gi0)

                    emit_wo(*prev_wo)

    nc.compile()
    return nc


def _deinterleave(w):
    """[64, DIM] interleaved rope rows -> [ev(32) | od(32)]."""
    return np.concatenate([w[0::2], w[1::2]], axis=0)


def shard_inputs(x, freqs_cis, wq, wkv_a, wkv_b, wo, kv_norm_w,
                 s_per_b=S, n_cores=N_CORES):
    bf16 = ml_dtypes.bfloat16
    scale = np.float32(QK ** -0.5)

    xf = np.asarray(x, np.float32).reshape(B * s_per_b, DIM)
    xT = xf.T.astype(bf16)                                 # [DIM, NTOK]
    NCH = B * s_per_b // CH
    DT_ = DIM // P
    # chunk-major SBUF image: [NCH, P, DT, CH]
    xc = np.ascontiguousarray(
        xT.reshape(DT_, P, NCH, CH).transpose(2, 1, 0, 3))

    fc = np.asarray(freqs_cis, np.float32)
    cosT = np.ascontiguousarray(fc[:, :, 0].T)             # [32, S]
    sinT = np.ascontiguousarray(fc[:, :, 1].T)
    cs4 = np.concatenate([cosT, sinT, cosT, sinT], axis=0)     # [128, S]
    cs4sw = np.concatenate([sinT, cosT, sinT, cosT], axis=0)

    wqf = np.asarray(wq, np.float32)                       # [H*QK, DIM]
    wkva = np.asarray(wkv_a, np.float32)                   # [C+R, DIM]
    wkvb = np.asarray(wkv_b, np.float32).reshape(H, NOPE + V, C)
    wof = np.asarray(wo, np.float32)                       # [DIM, H*V]
    wn = np.asarray(kv_norm_w, np.float32)                 # [C]

    kpe_blk = np.concatenate(
        [_deinterleave(wkva[C:C + R]), np.zeros((R, DIM), np.float32)], axis=0)

    ii = np.arange(P)
    tri = np.where(ii[:, None] <= ii[None, :], 0.0, NEG).astype(np.float32)
    maskA = np.zeros((P, HL, 2, P), np.float32)
    maskA[:, :, 0, :] = tri[:, None, :]
    maskB = np.full((P, HL, 2, P), NEG, np.float32)
    maskB[:, :, 1, :] = tri[:, None, :]
    maskA = maskA.astype(bf16)
    maskB = maskB.astype(bf16)

    in_maps = []
    for c in range(n_cores):
        h0 = c * HL
        ckv0 = c * CH                       # this core's kv token chunk
        pos0 = (c % (B * s_per_b // CH // B)) * CH  # position within batch
        wq_c = wqf.reshape(H, QK, DIM)[h0:h0 + HL] * scale  # [HL, QK, DIM]
        ropes = np.concatenate(
            [_deinterleave(wq_c[h, NOPE:]) for h in range(HL)], axis=0)
        blkkv = [wkva[kc * P:(kc + 1) * P] for kc in range(CT)] + [kpe_blk]
        # [blk, m, DIM] -> SBUF image [P, DT, blk, m]
        wallkv = np.ascontiguousarray(
            np.stack(blkkv, axis=0).reshape(CT + 1, P, DT_, P)
            .transpose(3, 2, 0, 1)).astype(bf16)
        blocks = [ropes, wq_c[0, :NOPE], wq_c[1, :NOPE]]
        wall = np.ascontiguousarray(
            np.stack(blocks, axis=0).reshape(NBLK - CT - 1, P, DT_, P)
            .transpose(3, 2, 0, 1)).astype(bf16)

        b1 = (wkvb[h0:h0 + HL, :NOPE, :] * wn[None, None, :])  # [HL,128,C]
        wkvb1T = np.ascontiguousarray(
            b1.transpose(2, 0, 1).reshape(CT, P, HL, NOPE)
            .transpose(1, 0, 2, 3)).astype(bf16)
        b2 = (wkvb[h0:h0 + HL, NOPE:, :] * wn[None, None, :])  # [HL,V,C]
        wkvb2 = np.ascontiguousarray(
            b2.transpose(2, 0, 1).reshape(CT, P, HL, V)
            .transpose(1, 0, 2, 3)).astype(bf16)
        woT_c = np.ascontiguousarray(
            wof[:, h0 * V:(h0 + HL) * V].T.reshape(HL, V, DIM)
            .transpose(1, 0, 2)).astype(bf16)              # [V, HL, DIM]
        in_maps.append({
            "xc": xc,
            "xTkv": np.ascontiguousarray(xc[c]),
            "cs4kv": np.ascontiguousarray(cs4[:, pos0:pos0 + CH]),
            "cs4swkv": np.ascontiguousarray(cs4sw[:, pos0:pos0 + CH]),
            "wallkv": wallkv,
            "wall": wall,
            "wkvb1T": wkvb1T,
            "wkvb2": wkvb2,
            "woT": woT_c,
            "cs4": cs4,
            "cs4sw": cs4sw,
            "maskA": maskA,
            "maskB": maskB,
        })
    return in_maps


_NC_CACHE = {}


def get_nc(s_per_b=S):
    if s_per_b not in _NC_CACHE:
        _NC_CACHE[s_per_b] = build_nc(s_per_b)
    return _NC_CACHE[s_per_b]


def kernel(x, freqs_cis, wq, wkv_a, wkv_b, wo, kv_norm_w, trace=False):
    nc = get_nc(S)
    in_maps = shard_inputs(x, freqs_cis, wq, wkv_a, wkv_b, wo, kv_norm_w)
    res = bass_utils.run_bass_kernel_spmd(
        nc, in_maps, core_ids=list(range(N_CORES)), trace=trace)
    y = res.results[0]["y"].astype(np.float32)
    for i in range(1, N_CORES):
        y += res.results[i]["y"].astype(np.float32)
    out = y.reshape(B, S, DIM)
    if trace:
        kernel.last_exec_time_ns = res.exec_time_ns
        kernel.last_results = res
    return out
